# revision 33
# baseline (speedup 1.0000x reference)
"""Trainium2 Bass kernel for nn_Compressor (sparse_attention block compressor).

Math (reference):
  proj = x @ [W_kv; W_gate]^T            # [b*s, 2048]
  kv   = proj[:, :1024] + ape[s%4]       # blockwise (RATIO=4) abs-pos bias
  sc   = proj[:, 1024:]
  window(blk) = {prev blk rows, ch 0:512} + {cur blk rows, ch 512:1024}
  pooled[blk, c] = softmax-gated channelwise pool over the 2*RATIO window
  out = (RMSNorm(pooled) -> rope on ch 448:512) @ H  (512x512 Hadamard)

Distribution: 8 cores, data-parallel over (batch, seq-half). Each core owns
2048 seq rows = 512 blocks; the 1-block halo is handled by shifting the
matmul rhs window by 4 rows (xs input carries 16 halo rows).

Key implementation tricks:
  * x^T in bf16 obtained directly by truncating f32 to the hi-16 bits.
  * Projections: W^T tiles stationary (lhsT), x^T moving -> PSUM layout
    [channels(part), m(free)], so the whole softmax pooling is free-axis
    DVE/ACT work and the halo is a free-axis slice offset.
  * Softmax without max-subtraction (scores are ~N(0,1.3); fp32 exp cannot
    overflow; block-0 masking is a 0/1 multiply on exp with a per-core mask).
  * RMSNorm channel reduction via a ones-vector matmul; the scale is applied
    per-partition after the final Hadamard matmul (everything in between is
    linear).
  * Rope pair swap via a tiny permutation matmul; cos/sin tables precomputed
    on host per core.

Schedule (the projection matmul stream is the ~458us critical resource; all
scalar/vector/DMA work hides under it):
  * PE warm-up matmuls during the initial DMA fill (clock ramp); batched
    4-chunk x DMAs via rearranged DRAM APs (one trigger instead of four).
  * Each chunk's RMSNorm/rope/Hadamard final phase is DEFERRED into the next
    chunk's instruction stream, after its first projection, so the in-order
    PE queue never waits on the pooling chain mid-stream.
  * Groups processed (3,0,1,2), gates before kv inside a group, so each
    post-projection chain ends on cheap DVE kv work; f16 staging for exp/kv;
    reciprocal_approx_fast for softmax denom and norm scale; Square and the
    scaled output copy on DVE so the ACT engine only switches tables between
    Exp and Sqrt (with a dummy-Sqrt preload for the tail).
  * bf16 tail matmuls (Hadamard/norm-sum/rope-swap), Hadamard emitted before
    the norm-sum so it overlaps the scale chain, and a PE-transpose (not
    DMA) for the [1,128]->[128,1] norm scale.
  * The very last kv projection runs in two 256-col halves with its drain +
    pooling pipelined against the second half; sq/rope for earlier groups
    are precomputed under it, leaving a ~4us serial tail.
"""

import os
import numpy as np
import ml_dtypes

import concourse.bass as bass
import concourse.bacc as bacc
import concourse.mybir as mybir
from concourse.tile import TileContext
from concourse.bass_utils import run_bass_kernel_spmd

BF16 = ml_dtypes.bfloat16
F32 = mybir.dt.float32
F16 = mybir.dt.float16
BF = mybir.dt.bfloat16

N_CORES = 8
DIM = 4096
OCH = 2048          # kv 1024 + gate 1024
ROWS = 2048         # own rows per core
XS_ROWS = 2064      # 16 halo/pad rows + 2048
MCH = 4             # m-chunks per core
MROWS = 512         # rows per m-chunk
NBLK = 128          # blocks per m-chunk
DCH = 32            # d chunks of 128
OCHK = 16           # o chunks of 128
# o-chunks 0..3 kv-first(prev), 4..7 kv-second(cur), 8..11 sc-first, 12..15 sc-second
FIRST_HALF = (0, 1, 2, 3, 8, 9, 10, 11)
GROUP_ORDER = (3, 0, 1, 2)   # group 3 first so rope overlaps; 2 last

_CACHE = {}


def _build():
    nc = bacc.Bacc("TRN2", target_bir_lowering=False, debug=False,
                   num_devices=N_CORES)
    xs = nc.dram_tensor("xs", [DIM, XS_ROWS], BF, kind="ExternalInput")
    wp = nc.dram_tensor("wp", [OCHK, 128, DCH, 128], BF, kind="ExternalInput")
    ape_d = nc.dram_tensor("ape_t", [128, 32], F32, kind="ExternalInput")
    cos_d = nc.dram_tensor("cos_t", [128, 512], F32, kind="ExternalInput")
    sin_d = nc.dram_tensor("sin_t", [128, 512], F32, kind="ExternalInput")
    psw_d = nc.dram_tensor("psw", [128, 128], BF, kind="ExternalInput")
    h_d = nc.dram_tensor("hmat", [128, 4, 512], BF, kind="ExternalInput")
    zmask_d = nc.dram_tensor("zmask", [128, 1], F32, kind="ExternalInput")
    out_d = nc.dram_tensor("out", [4 * NBLK, 512], F32, kind="ExternalOutput")

    with TileContext(nc) as tc:
        with (
            tc.tile_pool(name="const", bufs=1) as constp,
            tc.tile_pool(name="xt", bufs=2) as xtp,
            tc.tile_pool(name="wt", bufs=4) as wtp,
            tc.tile_pool(name="sb", bufs=2) as sbp,
            tc.tile_pool(name="pl", bufs=2) as plp,
            tc.tile_pool(name="sm", bufs=2) as smp,
            tc.tile_pool(name="osb", bufs=2) as outp,
            tc.tile_pool(name="proj", bufs=4, space="PSUM") as projp,
            tc.tile_pool(name="had", bufs=2, space="PSUM") as hadp,
            tc.tile_pool(name="aux", bufs=1, space="PSUM") as auxp,
        ):
            # ---- PE warm-up: ramp the tensor clock while DMAs fill ----
            wu_w = constp.tile([128, 128], BF, tag="wu_w")
            nc.vector.memset(wu_w[:], 0.0)
            wu_x = constp.tile([128, 512], BF, tag="wu_x")
            nc.vector.memset(wu_x[:], 0.0)
            wu_ps = projp.tile([128, 512], F32, tag="proj")
            for i in range(14):
                nc.tensor.matmul(wu_ps[:], lhsT=wu_w[:], rhs=wu_x[:],
                                 start=(i == 0), stop=(i == 13))

            # ---- constants (vector-engine DMA queue: off the x/W path) ----
            ape_sb = constp.tile([128, 32], F32, tag="ape")
            nc.scalar.dma_start(out=ape_sb[:], in_=ape_d[:, :])
            cos_sb = constp.tile([128, 512], F32, tag="cos")
            nc.scalar.dma_start(out=cos_sb[:], in_=cos_d[:, :])
            sin_sb = constp.tile([128, 512], F32, tag="sin")
            nc.scalar.dma_start(out=sin_sb[:], in_=sin_d[:, :])
            psw_sb = constp.tile([128, 128], BF, tag="psw")
            nc.scalar.dma_start(out=psw_sb[:], in_=psw_d[:, :])
            h_sb = constp.tile([128, 4, 512], BF, tag="h")
            nc.scalar.dma_start(out=h_sb[:], in_=h_d[:, :, :])
            zmask_sb = constp.tile([128, 1], F32, tag="zmask")
            nc.scalar.dma_start(out=zmask_sb[:], in_=zmask_d[:, :])
            ones_sb = constp.tile([128, 1], BF, tag="ones")
            nc.vector.memset(ones_sb[:], 1.0)
            one_sb = constp.tile([1, 1], F32, tag="one")
            nc.vector.memset(one_sb[:], 1.0)
            eps_sb = constp.tile([128, 1], F32, tag="eps")
            nc.vector.memset(eps_sb[:], 1e-6)

            X = mybir.AxisListType.X

            # uneven m-chunks: small first chunk so the PE starts on a short
            # x-DMA; small last chunk so the serial tail chain is short.
            CNB = (128, 128, 128, 128)               # blocks per chunk
            CB0 = (0, 128, 256, 384)                 # first block of chunk
            NCH = len(CNB)

            def g4(tile_ap):
                return tile_ap.rearrange("p (b r) -> p b r", r=4)

            def xd(xt, d):
                return xt[d // 16][:, d % 16, :]

            def emit_rope(c, pooled):
                """rope on chunk 3 (channels 384..511; rows 64.. are rope)"""
                nb = CNB[c]
                b0 = CB0[c]
                sw_ps = auxp.tile([128, NBLK], F32, tag="swap")
                nc.tensor.matmul(sw_ps[:, :nb], lhsT=psw_sb[:],
                                 rhs=pooled[:, 3, :nb],
                                 start=True, stop=True)
                cslice = cos_sb[:, b0:b0 + nb]
                sslice = sin_sb[:, b0:b0 + nb]
                tmpc = smp.tile([128, NBLK], F32, tag="tmpc")
                nc.vector.tensor_mul(tmpc[:, :nb], pooled[:, 3, :nb], cslice)
                tmps = smp.tile([128, NBLK], F32, tag="tmps")
                nc.vector.tensor_mul(tmps[:, :nb], sw_ps[:, :nb], sslice)
                nc.vector.tensor_add(pooled[:, 3, :nb], tmpc[:, :nb],
                                     tmps[:, :nb])

            def final_phase(c, pooled, sq=None):
                """RMSNorm scale + Hadamard + store for one m-chunk.
                Emitted AFTER the next chunk's first projection so the PE
                never waits on the pooling chain mid-stream. For the last
                chunk sq groups 3,0,1 are pre-filled; only group 2 remains."""
                nb = CNB[c]
                b0 = CB0[c]
                if sq is None:
                    emit_rope(c, pooled)
                    sq = sbp.tile([128, 4, NBLK], BF, tag="sq")
                    nc.vector.tensor_mul(sq[:, :, :nb], pooled[:, :, :nb],
                                         pooled[:, :, :nb])
                else:
                    nc.vector.tensor_mul(sq[:, 2, :nb], pooled[:, 2, :nb],
                                         pooled[:, 2, :nb])
                # Hadamard first on the PE: it needs only pooled+rope,
                # so it overlaps the norm-scale chain on ACT/DVE
                had_ps = hadp.tile([128, 512], F32, tag="had")
                for jj in range(4):
                    nc.tensor.matmul(had_ps[:nb, :], lhsT=pooled[:, jj, :nb],
                                     rhs=h_sb[:, jj, :],
                                     start=(jj == 0), stop=(jj == 3))
                ns_ps = auxp.tile([1, 4, NBLK], F32, tag="ns")
                nc.tensor.matmul(
                    ns_ps[0:1, :, :nb],
                    lhsT=ones_sb[:, 0:1],
                    rhs=sq[:, :, :nb],
                    start=True, stop=True)
                var_row = smp.tile([1, NBLK], F32, tag="var_row")
                nc.vector.reduce_sum(
                    var_row[:, :nb],
                    ns_ps[0:1, :, :nb].rearrange("p c b -> p b c"),
                    axis=X)
                sd_row = smp.tile([1, NBLK], F32, tag="sd_row")
                nc.scalar.activation(sd_row[:, :nb], var_row[:, :nb],
                                     mybir.ActivationFunctionType.Sqrt,
                                     scale=1.0 / 512.0, bias=eps_sb[0:1, 0:1])
                scale_row = smp.tile([1, NBLK], F32, tag="scale_row")
                nc.vector.reciprocal_approx_fast(scale_row[:, :nb],
                                                 sd_row[:, :nb])
                # transpose [1,nb] -> [nb,1] on the PE (DMA transpose is slow)
                sc_ps = auxp.tile([128, NBLK], F32, tag="swap")
                nc.tensor.matmul(sc_ps[:nb, 0:1], lhsT=scale_row[0:1, :nb],
                                 rhs=one_sb[0:1, 0:1], start=True, stop=True)
                scale_col = smp.tile([128, 1], F32, tag="scale_col")
                nc.vector.tensor_scalar_mul(scale_col[:nb, :],
                                            sc_ps[:nb, 0:1], 1.0)
                out_sb = outp.tile([128, 512], F32, tag="out")
                nc.vector.tensor_scalar_mul(out_sb[:nb, :], had_ps[:nb, :],
                                            scale_col[:nb, 0:1])
                nc.sync.dma_start(
                    out=out_d[b0:b0 + nb, :], in_=out_sb[:nb, :])

            def emit_x_half(c, half, dual):
                """Issue one x^T half-tile (16 d-chunks). DMA-dependency
                thresholds snapshot the queue counter at consumer emission,
                so emitting half B only after pass-A consumers lets pass A
                start once half A lands. `dual` splits the triggers across
                two queues (each queue's transfers serialize; two engines
                run in parallel)."""
                cs = 4 * CNB[c] + 16
                r0 = 4 * CB0[c]
                xth = xtp.tile([128, DCH // 2, 528], BF,
                               tag=("xtA", "xtB")[half])
                engs = (nc.sync, nc.scalar) if dual else (nc.sync, nc.sync)
                for i, g in enumerate(range(0, DCH // 2, 4)):
                    gg = g + half * (DCH // 2)
                    engs[i % 2].dma_start(
                        out=xth[:, g:g + 4, :cs],
                        in_=xs[128 * gg:128 * (gg + 4), r0:r0 + cs]
                        .rearrange("(c p) m -> p c m", p=128))
                return xth

            def emit_x(c):
                return (emit_x_half(c, 0, False), emit_x_half(c, 1, False))

            pending = None  # (chunk, pooled) awaiting its final phase
            xt_next = list(emit_x(0))
            for c in range(NCH):
                nb = CNB[c]
                mr = 4 * nb
                xt = xt_next
                group = {}  # role -> sbuf tile for the current group j
                pooled = plp.tile([128, 4, NBLK], BF, tag="pooled")
                for gi, j in enumerate(GROUP_ORDER):
                    # gates first: the post-projection pooling chain then
                    # ends on cheap DVE kv work, not the exp ACT chain
                    prog = (c == NCH - 1 and gi == 3)
                    for t, oc in enumerate((j + 8, j + 12, j, j + 4)):
                        w = wtp.tile([128, DCH, 128], BF, tag="w")
                        nc.gpsimd.dma_start(out=w[:], in_=wp[oc])
                        ps = projp.tile([128, MROWS], F32, tag="proj")
                        off = 12 if oc in FIRST_HALF else 16
                        if prog and t == 3:
                            break
                        for d in range(DCH):
                            nc.tensor.matmul(
                                ps[:, :mr],
                                lhsT=w[:, d, :],
                                rhs=xd(xt, d)[:, off:off + mr],
                                start=(d == 0),
                                stop=(d == DCH - 1),
                            )
                        if oc < 8:
                            # kv chunk: PSUM -> SBUF f16 with ape bias added
                            kv = sbp.tile([128, MROWS], F16, tag=f"kv{t - 2}")
                            a = oc  # ape chunk = kv o-chunk (0..7)
                            ape_ap = (ape_sb[:, 4 * a:4 * a + 4]
                                      .unsqueeze(1).to_broadcast((128, nb, 4)))
                            nc.vector.tensor_add(
                                kv[:, :mr].rearrange("p (b r) -> p b r", r=4),
                                ps[:, :mr].rearrange("p (b r) -> p b r", r=4),
                                ape_ap,
                            )
                            group[f"kv{t - 2}"] = kv
                        else:
                            # score chunk: e = exp(psum) straight to SBUF f16
                            e = sbp.tile([128, MROWS], F16, tag=f"e{t}")
                            assert t < 2
                            nc.scalar.activation(
                                e[:, :mr], ps[:, :mr],
                                mybir.ActivationFunctionType.Exp)
                            if c == 0 and oc < 12:
                                # block-0 of even cores: zero the 4 prev-window
                                # weights (zmask = 0 even / 1 odd)
                                nc.vector.tensor_scalar_mul(
                                    e[:, 0:4], e[:, 0:4], zmask_sb[:, 0:1])
                            group[f"e{t}"] = e

                    if gi == 0 and pending is not None:
                        final_phase(*pending)
                        pending = None
                    if gi == 1 and c == NCH - 1:
                        # last chunk: rope early (group 3 pooled is ready;
                        # the PE reaches it behind group-0's projections)
                        emit_rope(c, pooled)
                    if gi == 2 and c + 1 < NCH:
                        xt_next = emit_x(c + 1)

                    kv1 = group["kv0"]
                    e1, e2 = group["e0"], group["e1"]

                    s1 = smp.tile([128, NBLK], F32, tag="s1")
                    nc.vector.reduce_sum(s1[:, :nb], g4(e1[:, :mr]), axis=X)
                    s2 = smp.tile([128, NBLK], F32, tag="s2")
                    nc.vector.reduce_sum(s2[:, :nb], g4(e2[:, :mr]), axis=X)
                    ssum = smp.tile([128, NBLK], F32, tag="ssum")
                    nc.vector.tensor_add(ssum[:, :nb], s1[:, :nb], s2[:, :nb])
                    rinv = smp.tile([128, NBLK], F32, tag="rinv")
                    nc.vector.reciprocal_approx_fast(rinv[:, :nb],
                                                     ssum[:, :nb])
                    pm = sbp.tile([128, MROWS], F32, tag="pm")
                    nc.vector.tensor_mul(pm[:, :mr], e1[:, :mr], kv1[:, :mr])
                    q1 = smp.tile([128, NBLK], F32, tag="q1")
                    nc.vector.reduce_sum(q1[:, :nb], g4(pm[:, :mr]), axis=X)

                    if prog:
                        # last group of the last chunk: the final kv
                        # projection runs in halves, with its drain and
                        # pooling pipelined against the second half.
                        h = mr // 2
                        nb2 = nb // 2
                        oc = j + 4
                        sq_t = sbp.tile([128, 4, NBLK], BF, tag="sq")
                        kv = sbp.tile([128, MROWS], F16, tag="kv1")
                        pm2 = sbp.tile([128, MROWS], F32, tag="pm2")
                        q2 = smp.tile([128, NBLK], F32, tag="q2")
                        qsum = smp.tile([128, NBLK], F32, tag="qsum")
                        a = oc
                        for hi, (c0, c1, b0h, b1h) in enumerate(
                                ((0, h, 0, nb2), (h, mr, nb2, nb))):
                            for d in range(DCH):
                                nc.tensor.matmul(
                                    ps[:, c0:c1], lhsT=w[:, d, :],
                                    rhs=xd(xt, d)[:, off + c0:off + c1],
                                    start=(d == 0), stop=(d == DCH - 1),
                                    skip_group_check=True)
                            ape_ap = (ape_sb[:, 4 * a:4 * a + 4]
                                      .unsqueeze(1)
                                      .to_broadcast((128, nb2, 4)))
                            nc.vector.tensor_add(
                                kv[:, c0:c1].rearrange(
                                    "p (b r) -> p b r", r=4),
                                ps[:, c0:c1].rearrange(
                                    "p (b r) -> p b r", r=4),
                                ape_ap)
                            nc.vector.tensor_mul(pm2[:, c0:c1], e2[:, c0:c1],
                                                 kv[:, c0:c1])
                            nc.vector.reduce_sum(q2[:, b0h:b1h],
                                                 g4(pm2[:, c0:c1]), axis=X)
                            nc.vector.tensor_add(qsum[:, b0h:b1h],
                                                 q1[:, b0h:b1h],
                                                 q2[:, b0h:b1h])
                            nc.vector.tensor_mul(pooled[:, j, b0h:b1h],
                                                 qsum[:, b0h:b1h],
                                                 rinv[:, b0h:b1h])
                            if hi == 0:
                                # sq for the finished groups + Sqrt preload,
                                # hidden under the second-half projections
                                for jj in (3, 0, 1):
                                    nc.vector.tensor_mul(
                                        sq_t[:, jj, :nb], pooled[:, jj, :nb],
                                        pooled[:, jj, :nb])
                                warm = smp.tile([1, 1], F32, tag="sqwarm")
                                nc.scalar.activation(
                                    warm[:], eps_sb[0:1, 0:1],
                                    mybir.ActivationFunctionType.Sqrt,
                                    scale=1.0, bias=eps_sb[0:1, 0:1])
                    else:
                        kv2 = group["kv1"]
                        pm2 = sbp.tile([128, MROWS], F32, tag="pm2")
                        nc.vector.tensor_mul(pm2[:, :mr], e2[:, :mr],
                                             kv2[:, :mr])
                        q2 = smp.tile([128, NBLK], F32, tag="q2")
                        nc.vector.reduce_sum(q2[:, :nb], g4(pm2[:, :mr]),
                                             axis=X)
                        qsum = smp.tile([128, NBLK], F32, tag="qsum")
                        nc.vector.tensor_add(qsum[:, :nb], q1[:, :nb],
                                             q2[:, :nb])
                        nc.vector.tensor_mul(pooled[:, j, :nb], qsum[:, :nb],
                                             rinv[:, :nb])

                pending = (c, pooled)

            final_phase(*pending, sq_t)
    nc.compile()
    return nc


def _prep_shared(W_kv, W_gate, ape, norm_w, H):
    W = np.concatenate([W_kv, W_gate], axis=0).astype(np.float32)  # [2048, 4096]
    Wb = W.astype(BF16)
    wp = np.ascontiguousarray(
        Wb.T.reshape(DCH, 128, OCHK, 128).transpose(2, 1, 0, 3))  # [16,128,32,128]
    ape_t = np.ascontiguousarray(
        ape.astype(np.float32).T.reshape(8, 128, 4).transpose(1, 0, 2)
    ).reshape(128, 32)
    psw = np.zeros((128, 128), np.float32)
    idx = np.arange(64)
    psw[idx, idx] = 1.0
    k2 = np.arange(0, 64, 2)
    psw[64 + k2 + 1, 64 + k2] = 1.0
    psw[64 + k2, 64 + k2 + 1] = 1.0
    hm = np.ascontiguousarray(
        (norm_w.astype(np.float32)[:, None] * H.astype(np.float32))
        .reshape(4, 128, 512).transpose(1, 0, 2)).astype(BF16)
    return wp, ape_t, psw.astype(BF16), hm


def _hadamard(n):
    h = np.array([[1.0]], dtype=np.float32)
    while h.shape[0] < n:
        h = np.block([[h, h], [h, -h]])
    return (h / np.sqrt(n)).astype(np.float32)


def _make_in_maps(x, W_kv, W_gate, ape, norm_w, freqs_cis):
    b, s, _ = x.shape
    H = _hadamard(512)
    wp, ape_t, psw, hm = _prep_shared(W_kv, W_gate, ape, norm_w, H)

    # truncate-to-bf16 (hi-16 planes of the f32 words) and transpose once
    xh = x.reshape(b * s, DIM).view(BF16)[:, 1::2]
    xT = np.ascontiguousarray(xh.T)  # [4096, 16384]
    fr = freqs_cis[:, :, 0]  # [nb, 32]
    fi = freqs_cis[:, :, 1]

    in_maps = []
    for c in range(N_CORES):
        batch, half = c // 2, c % 2
        R0 = batch * s + half * ROWS
        xs = np.zeros((DIM, XS_ROWS), BF16)
        xs[:, 16:] = xT[:, R0:R0 + ROWS]
        if half == 1:
            xs[:, :16] = xT[:, R0 - 16:R0]

        g0 = half * 512
        bi = np.arange(g0, g0 + 512)
        cos_t = np.zeros((128, 512), np.float32)
        cos_t[:64] = 1.0
        cos_t[64:] = np.repeat(fr[bi].T, 2, axis=0)
        sin_t = np.zeros((128, 512), np.float32)
        st = np.repeat(fi[bi].T, 2, axis=0)
        st[0::2] *= -1.0
        sin_t[64:] = st

        zmask = np.full((128, 1), 0.0 if half == 0 else 1.0, np.float32)
        in_maps.append({
            "xs": xs, "wp": wp, "ape_t": ape_t,
            "cos_t": cos_t, "sin_t": sin_t, "psw": psw,
            "hmat": hm, "zmask": zmask,
        })
    return in_maps


def kernel(x, W_kv, W_gate, ape, norm_w, freqs_cis, start_pos=0):
    x = np.asarray(x, dtype=np.float32)
    W_kv = np.asarray(W_kv, dtype=np.float32)
    W_gate = np.asarray(W_gate, dtype=np.float32)
    ape = np.asarray(ape, dtype=np.float32)
    norm_w = np.asarray(norm_w, dtype=np.float32)
    freqs_cis = np.asarray(freqs_cis, dtype=np.float32)

    b, s, _ = x.shape
    nb = s // 4
    assert (b, s) == (4, 4096), (b, s)

    if "nc" not in _CACHE:
        _CACHE["nc"] = _build()
    nc = _CACHE["nc"]

    in_maps = _make_in_maps(x, W_kv, W_gate, ape, norm_w, freqs_cis)

    trace = os.environ.get("KERNEL_TRACE", "") not in ("", "0")
    res = run_bass_kernel_spmd(nc, in_maps, core_ids=list(range(N_CORES)),
                               trace=trace)
    kernel.last_results = res
    out = np.concatenate([res.results[c]["out"] for c in range(N_CORES)], axis=0)
    return np.ascontiguousarray(out.reshape(b, nb, 512))


# revision 34
# speedup vs baseline: 1.0072x; 1.0072x over previous
"""Trainium2 Bass kernel for nn_Compressor (sparse_attention block compressor).

Math (reference):
  proj = x @ [W_kv; W_gate]^T            # [b*s, 2048]
  kv   = proj[:, :1024] + ape[s%4]       # blockwise (RATIO=4) abs-pos bias
  sc   = proj[:, 1024:]
  window(blk) = {prev blk rows, ch 0:512} + {cur blk rows, ch 512:1024}
  pooled[blk, c] = softmax-gated channelwise pool over the 2*RATIO window
  out = (RMSNorm(pooled) -> rope on ch 448:512) @ H  (512x512 Hadamard)

Distribution: 8 cores, data-parallel over (batch, seq-half). Each core owns
2048 seq rows = 512 blocks; the 1-block halo is handled by shifting the
matmul rhs window by 4 rows (xs input carries 16 halo rows).

Key implementation tricks:
  * x^T in bf16 obtained directly by truncating f32 to the hi-16 bits.
  * Projections: W^T tiles stationary (lhsT), x^T moving -> PSUM layout
    [channels(part), m(free)], so the whole softmax pooling is free-axis
    DVE/ACT work and the halo is a free-axis slice offset.
  * Softmax without max-subtraction (scores are ~N(0,1.3); fp32 exp cannot
    overflow; block-0 masking is a 0/1 multiply on exp with a per-core mask).
  * RMSNorm channel reduction via a ones-vector matmul; the scale is applied
    per-partition after the final Hadamard matmul (everything in between is
    linear).
  * Rope pair swap via a tiny permutation matmul; cos/sin tables precomputed
    on host per core.

Schedule (the projection matmul stream is the ~458us critical resource; all
scalar/vector/DMA work hides under it):
  * PE warm-up matmuls during the initial DMA fill (clock ramp); batched
    4-chunk x DMAs via rearranged DRAM APs (one trigger instead of four),
    x in two half-tiles (DMA-dependency thresholds snapshot whole queues).
  * Each chunk's RMSNorm/rope/Hadamard final phase is DEFERRED into the next
    chunk's instruction stream, after its first projection, so the in-order
    PE queue never waits on the pooling chain mid-stream.
  * Groups processed (3,0,1,2), gates before kv inside a group, so each
    post-projection chain ends on cheap DVE kv work; f16 staging for exp/kv;
    reciprocal_approx_fast for softmax denom and norm scale; Square and the
    scaled output copy on DVE so the ACT engine only switches tables between
    Exp and Sqrt (with a dummy-Sqrt preload for the tail).
  * bf16 tail matmuls (Hadamard/norm-sum/rope-swap), Hadamard emitted before
    the norm-sum so it overlaps the scale chain, and a PE-transpose (not
    DMA) for the [1,128]->[128,1] norm scale.
  * The very last kv projection runs in two 256-col halves with its drain +
    pooling pipelined against the second half; sq/rope for earlier groups
    are precomputed under it, leaving a ~4us serial tail.
"""

import os
import numpy as np
import ml_dtypes

import concourse.bass as bass
import concourse.bacc as bacc
import concourse.mybir as mybir
from concourse.tile import TileContext
from concourse.bass_utils import run_bass_kernel_spmd

BF16 = ml_dtypes.bfloat16
F32 = mybir.dt.float32
F16 = mybir.dt.float16
BF = mybir.dt.bfloat16

N_CORES = 8
DIM = 4096
OCH = 2048          # kv 1024 + gate 1024
ROWS = 2048         # own rows per core
XS_ROWS = 2064      # 16 halo/pad rows + 2048
MCH = 4             # m-chunks per core
MROWS = 512         # rows per m-chunk
NBLK = 128          # blocks per m-chunk
DCH = 32            # d chunks of 128
OCHK = 16           # o chunks of 128
# o-chunks 0..3 kv-first(prev), 4..7 kv-second(cur), 8..11 sc-first, 12..15 sc-second
FIRST_HALF = (0, 1, 2, 3, 8, 9, 10, 11)
GROUP_ORDER = (3, 0, 1, 2)   # group 3 first so rope overlaps; 2 last

_CACHE = {}


def _build():
    nc = bacc.Bacc("TRN2", target_bir_lowering=False, debug=False,
                   num_devices=N_CORES)
    xs = nc.dram_tensor("xs", [DIM, XS_ROWS], BF, kind="ExternalInput")
    wp = nc.dram_tensor("wp", [OCHK, 128, DCH, 128], BF, kind="ExternalInput")
    ape_d = nc.dram_tensor("ape_t", [128, 32], F32, kind="ExternalInput")
    cos_d = nc.dram_tensor("cos_t", [128, 512], F32, kind="ExternalInput")
    sin_d = nc.dram_tensor("sin_t", [128, 512], F32, kind="ExternalInput")
    psw_d = nc.dram_tensor("psw", [128, 128], BF, kind="ExternalInput")
    h_d = nc.dram_tensor("hmat", [128, 4, 512], BF, kind="ExternalInput")
    zmask_d = nc.dram_tensor("zmask", [128, 1], F32, kind="ExternalInput")
    out_d = nc.dram_tensor("out", [4 * NBLK, 512], F32, kind="ExternalOutput")

    with TileContext(nc) as tc:
        with (
            tc.tile_pool(name="const", bufs=1) as constp,
            tc.tile_pool(name="xt", bufs=2) as xtp,
            tc.tile_pool(name="wt", bufs=4) as wtp,
            tc.tile_pool(name="sb", bufs=2) as sbp,
            tc.tile_pool(name="pl", bufs=2) as plp,
            tc.tile_pool(name="sm", bufs=2) as smp,
            tc.tile_pool(name="osb", bufs=2) as outp,
            tc.tile_pool(name="proj", bufs=4, space="PSUM") as projp,
            tc.tile_pool(name="had", bufs=2, space="PSUM") as hadp,
            tc.tile_pool(name="aux", bufs=1, space="PSUM") as auxp,
        ):
            # ---- PE warm-up: ramp the tensor clock while DMAs fill ----
            wu_w = constp.tile([128, 128], BF, tag="wu_w")
            nc.vector.memset(wu_w[:], 0.0)
            wu_x = constp.tile([128, 512], BF, tag="wu_x")
            nc.vector.memset(wu_x[:], 0.0)
            wu_ps = projp.tile([128, 512], F32, tag="proj")
            for i in range(14):
                nc.tensor.matmul(wu_ps[:], lhsT=wu_w[:], rhs=wu_x[:],
                                 start=(i == 0), stop=(i == 13))

            # ---- constants (vector-engine DMA queue: off the x/W path) ----
            ape_sb = constp.tile([128, 32], F32, tag="ape")
            nc.scalar.dma_start(out=ape_sb[:], in_=ape_d[:, :])
            cos_sb = constp.tile([128, 512], F32, tag="cos")
            nc.scalar.dma_start(out=cos_sb[:], in_=cos_d[:, :])
            sin_sb = constp.tile([128, 512], F32, tag="sin")
            nc.scalar.dma_start(out=sin_sb[:], in_=sin_d[:, :])
            psw_sb = constp.tile([128, 128], BF, tag="psw")
            nc.scalar.dma_start(out=psw_sb[:], in_=psw_d[:, :])
            h_sb = constp.tile([128, 4, 512], BF, tag="h")
            nc.scalar.dma_start(out=h_sb[:], in_=h_d[:, :, :])
            zmask_sb = constp.tile([128, 1], F32, tag="zmask")
            nc.scalar.dma_start(out=zmask_sb[:], in_=zmask_d[:, :])
            ones_sb = constp.tile([128, 1], BF, tag="ones")
            nc.vector.memset(ones_sb[:], 1.0)
            one_sb = constp.tile([1, 1], F32, tag="one")
            nc.vector.memset(one_sb[:], 1.0)
            eps_sb = constp.tile([128, 1], F32, tag="eps")
            nc.vector.memset(eps_sb[:], 1e-6)

            X = mybir.AxisListType.X

            # uneven m-chunks: small first chunk so the PE starts on a short
            # x-DMA; small last chunk so the serial tail chain is short.
            CNB = (128, 128, 128, 128)               # blocks per chunk
            CB0 = (0, 128, 256, 384)                 # first block of chunk
            NCH = len(CNB)

            def g4(tile_ap):
                return tile_ap.rearrange("p (b r) -> p b r", r=4)

            def xd(xt, d):
                return xt[d // 16][:, d % 16, :]

            def emit_rope(c, pooled):
                """rope on chunk 3 (channels 384..511; rows 64.. are rope)"""
                nb = CNB[c]
                b0 = CB0[c]
                sw_ps = auxp.tile([128, NBLK], F32, tag="swap")
                nc.tensor.matmul(sw_ps[:, :nb], lhsT=psw_sb[:],
                                 rhs=pooled[:, 3, :nb],
                                 start=True, stop=True)
                cslice = cos_sb[:, b0:b0 + nb]
                sslice = sin_sb[:, b0:b0 + nb]
                tmpc = smp.tile([128, NBLK], F32, tag="tmpc")
                nc.vector.tensor_mul(tmpc[:, :nb], pooled[:, 3, :nb], cslice)
                tmps = smp.tile([128, NBLK], F32, tag="tmps")
                nc.vector.tensor_mul(tmps[:, :nb], sw_ps[:, :nb], sslice)
                nc.vector.tensor_add(pooled[:, 3, :nb], tmpc[:, :nb],
                                     tmps[:, :nb])

            def final_phase(c, pooled, sq=None):
                """RMSNorm scale + Hadamard + store for one m-chunk.
                Emitted AFTER the next chunk's first projection so the PE
                never waits on the pooling chain mid-stream. For the last
                chunk sq groups 3,0,1 are pre-filled; only group 2 remains."""
                nb = CNB[c]
                b0 = CB0[c]
                if sq is None:
                    emit_rope(c, pooled)
                    sq = sbp.tile([128, 4, NBLK], BF, tag="sq")
                    nc.vector.tensor_mul(sq[:, :, :nb], pooled[:, :, :nb],
                                         pooled[:, :, :nb])
                else:
                    nc.vector.tensor_mul(sq[:, 2, :nb], pooled[:, 2, :nb],
                                         pooled[:, 2, :nb])
                # Hadamard first on the PE: it needs only pooled+rope,
                # so it overlaps the norm-scale chain on ACT/DVE
                had_ps = hadp.tile([128, 512], F32, tag="had")
                for jj in range(4):
                    nc.tensor.matmul(had_ps[:nb, :], lhsT=pooled[:, jj, :nb],
                                     rhs=h_sb[:, jj, :],
                                     start=(jj == 0), stop=(jj == 3))
                ns_ps = auxp.tile([1, 4, NBLK], F32, tag="ns")
                nc.tensor.matmul(
                    ns_ps[0:1, :, :nb],
                    lhsT=ones_sb[:, 0:1],
                    rhs=sq[:, :, :nb],
                    start=True, stop=True)
                var_row = smp.tile([1, NBLK], F32, tag="var_row")
                nc.vector.reduce_sum(
                    var_row[:, :nb],
                    ns_ps[0:1, :, :nb].rearrange("p c b -> p b c"),
                    axis=X)
                sd_row = smp.tile([1, NBLK], F32, tag="sd_row")
                nc.scalar.activation(sd_row[:, :nb], var_row[:, :nb],
                                     mybir.ActivationFunctionType.Sqrt,
                                     scale=1.0 / 512.0, bias=eps_sb[0:1, 0:1])
                scale_row = smp.tile([1, NBLK], F32, tag="scale_row")
                nc.vector.reciprocal_approx_fast(scale_row[:, :nb],
                                                 sd_row[:, :nb])
                # transpose [1,nb] -> [nb,1] on the PE (DMA transpose is slow)
                sc_ps = auxp.tile([128, NBLK], F32, tag="swap")
                nc.tensor.matmul(sc_ps[:nb, 0:1], lhsT=scale_row[0:1, :nb],
                                 rhs=one_sb[0:1, 0:1], start=True, stop=True)
                scale_col = smp.tile([128, 1], F32, tag="scale_col")
                nc.vector.tensor_scalar_mul(scale_col[:nb, :],
                                            sc_ps[:nb, 0:1], 1.0)
                out_sb = outp.tile([128, 512], F32, tag="out")
                nc.vector.tensor_scalar_mul(out_sb[:nb, :], had_ps[:nb, :],
                                            scale_col[:nb, 0:1])
                nc.sync.dma_start(
                    out=out_d[b0:b0 + nb, :], in_=out_sb[:nb, :])

            def emit_x_half(c, half, dual):
                """Issue one x^T half-tile (16 d-chunks). DMA-dependency
                thresholds snapshot the queue counter at consumer emission,
                so emitting half B only after pass-A consumers lets pass A
                start once half A lands. `dual` splits the triggers across
                two queues (each queue's transfers serialize; two engines
                run in parallel)."""
                cs = 4 * CNB[c] + 16
                r0 = 4 * CB0[c]
                xth = xtp.tile([128, DCH // 2, 528], BF,
                               tag=("xtA", "xtB")[half])
                engs = (nc.sync, nc.scalar) if dual else (nc.sync, nc.sync)
                for i, g in enumerate(range(0, DCH // 2, 4)):
                    gg = g + half * (DCH // 2)
                    engs[i % 2].dma_start(
                        out=xth[:, g:g + 4, :cs],
                        in_=xs[128 * gg:128 * (gg + 4), r0:r0 + cs]
                        .rearrange("(c p) m -> p c m", p=128))
                return xth

            def emit_x(c):
                return (emit_x_half(c, 0, False), emit_x_half(c, 1, False))

            pending = None  # (chunk, pooled) awaiting its final phase
            xt_next = list(emit_x(0))
            for c in range(NCH):
                nb = CNB[c]
                mr = 4 * nb
                xt = xt_next
                group = {}  # role -> sbuf tile for the current group j
                pooled = plp.tile([128, 4, NBLK], BF, tag="pooled")
                for gi, j in enumerate(GROUP_ORDER):
                    # gates first: the post-projection pooling chain then
                    # ends on cheap DVE kv work, not the exp ACT chain
                    prog = (c == NCH - 1 and gi == 3)
                    for t, oc in enumerate((j + 8, j + 12, j, j + 4)):
                        w = wtp.tile([128, DCH, 128], BF, tag="w")
                        nc.gpsimd.dma_start(out=w[:], in_=wp[oc])
                        ps = projp.tile([128, MROWS], F32, tag="proj")
                        off = 12 if oc in FIRST_HALF else 16
                        if prog and t == 3:
                            break
                        for d in range(DCH):
                            nc.tensor.matmul(
                                ps[:, :mr],
                                lhsT=w[:, d, :],
                                rhs=xd(xt, d)[:, off:off + mr],
                                start=(d == 0),
                                stop=(d == DCH - 1),
                            )
                        if oc < 8:
                            # kv chunk: PSUM -> SBUF f16 with ape bias added
                            kv = sbp.tile([128, MROWS], F16, tag=f"kv{t - 2}")
                            a = oc  # ape chunk = kv o-chunk (0..7)
                            ape_ap = (ape_sb[:, 4 * a:4 * a + 4]
                                      .unsqueeze(1).to_broadcast((128, nb, 4)))
                            nc.vector.tensor_add(
                                kv[:, :mr].rearrange("p (b r) -> p b r", r=4),
                                ps[:, :mr].rearrange("p (b r) -> p b r", r=4),
                                ape_ap,
                            )
                            group[f"kv{t - 2}"] = kv
                        else:
                            # score chunk: e = exp(psum) straight to SBUF f16
                            e = sbp.tile([128, MROWS], F16, tag=f"e{t}")
                            assert t < 2
                            nc.scalar.activation(
                                e[:, :mr], ps[:, :mr],
                                mybir.ActivationFunctionType.Exp)
                            if c == 0 and oc < 12:
                                # block-0 of even cores: zero the 4 prev-window
                                # weights (zmask = 0 even / 1 odd)
                                nc.vector.tensor_scalar_mul(
                                    e[:, 0:4], e[:, 0:4], zmask_sb[:, 0:1])
                            group[f"e{t}"] = e

                    if gi == 0 and pending is not None:
                        final_phase(*pending)
                        pending = None
                    if gi == 1 and c == NCH - 1:
                        # last chunk: rope early (group 3 pooled is ready;
                        # the PE reaches it behind group-0's projections)
                        emit_rope(c, pooled)
                    if gi == 2 and c + 1 < NCH:
                        xt_next = emit_x(c + 1)

                    kv1 = group["kv0"]
                    e1, e2 = group["e0"], group["e1"]

                    s1 = smp.tile([128, NBLK], F32, tag="s1")
                    nc.vector.reduce_sum(s1[:, :nb], g4(e1[:, :mr]), axis=X)
                    s2 = smp.tile([128, NBLK], F32, tag="s2")
                    nc.vector.reduce_sum(s2[:, :nb], g4(e2[:, :mr]), axis=X)
                    ssum = smp.tile([128, NBLK], F32, tag="ssum")
                    nc.vector.tensor_add(ssum[:, :nb], s1[:, :nb], s2[:, :nb])
                    rinv = smp.tile([128, NBLK], F32, tag="rinv")
                    nc.vector.reciprocal_approx_fast(rinv[:, :nb],
                                                     ssum[:, :nb])
                    pm = sbp.tile([128, MROWS], F32, tag="pm")
                    nc.vector.tensor_mul(pm[:, :mr], e1[:, :mr], kv1[:, :mr])
                    q1 = smp.tile([128, NBLK], F32, tag="q1")
                    nc.vector.reduce_sum(q1[:, :nb], g4(pm[:, :mr]), axis=X)

                    if prog:
                        # last group of the last chunk: the final kv
                        # projection runs in halves, with its drain and
                        # pooling pipelined against the second half.
                        h = mr // 2
                        nb2 = nb // 2
                        oc = j + 4
                        sq_t = sbp.tile([128, 4, NBLK], BF, tag="sq")
                        kv = sbp.tile([128, MROWS], F16, tag="kv1")
                        pm2 = sbp.tile([128, MROWS], F32, tag="pm2")
                        q2 = smp.tile([128, NBLK], F32, tag="q2")
                        qsum = smp.tile([128, NBLK], F32, tag="qsum")
                        a = oc
                        for hi, (c0, c1, b0h, b1h) in enumerate(
                                ((0, h, 0, nb2), (h, mr, nb2, nb))):
                            for d in range(DCH):
                                nc.tensor.matmul(
                                    ps[:, c0:c1], lhsT=w[:, d, :],
                                    rhs=xd(xt, d)[:, off + c0:off + c1],
                                    start=(d == 0), stop=(d == DCH - 1),
                                    skip_group_check=True)
                            ape_ap = (ape_sb[:, 4 * a:4 * a + 4]
                                      .unsqueeze(1)
                                      .to_broadcast((128, nb2, 4)))
                            nc.vector.tensor_add(
                                kv[:, c0:c1].rearrange(
                                    "p (b r) -> p b r", r=4),
                                ps[:, c0:c1].rearrange(
                                    "p (b r) -> p b r", r=4),
                                ape_ap)
                            nc.vector.tensor_mul(pm2[:, c0:c1], e2[:, c0:c1],
                                                 kv[:, c0:c1])
                            nc.vector.reduce_sum(q2[:, b0h:b1h],
                                                 g4(pm2[:, c0:c1]), axis=X)
                            nc.vector.tensor_add(qsum[:, b0h:b1h],
                                                 q1[:, b0h:b1h],
                                                 q2[:, b0h:b1h])
                            nc.vector.tensor_mul(pooled[:, j, b0h:b1h],
                                                 qsum[:, b0h:b1h],
                                                 rinv[:, b0h:b1h])
                            if hi == 0:
                                # sq for the finished groups + Sqrt preload,
                                # hidden under the second-half projections
                                for jj in (3, 0, 1):
                                    nc.vector.tensor_mul(
                                        sq_t[:, jj, :nb], pooled[:, jj, :nb],
                                        pooled[:, jj, :nb])
                                warm = smp.tile([1, 1], F32, tag="sqwarm")
                                nc.scalar.activation(
                                    warm[:], eps_sb[0:1, 0:1],
                                    mybir.ActivationFunctionType.Sqrt,
                                    scale=1.0, bias=eps_sb[0:1, 0:1])
                    else:
                        kv2 = group["kv1"]
                        pm2 = sbp.tile([128, MROWS], F32, tag="pm2")
                        nc.vector.tensor_mul(pm2[:, :mr], e2[:, :mr],
                                             kv2[:, :mr])
                        q2 = smp.tile([128, NBLK], F32, tag="q2")
                        nc.vector.reduce_sum(q2[:, :nb], g4(pm2[:, :mr]),
                                             axis=X)
                        qsum = smp.tile([128, NBLK], F32, tag="qsum")
                        nc.vector.tensor_add(qsum[:, :nb], q1[:, :nb],
                                             q2[:, :nb])
                        nc.vector.tensor_mul(pooled[:, j, :nb], qsum[:, :nb],
                                             rinv[:, :nb])

                pending = (c, pooled)

            final_phase(*pending, sq_t)
    nc.compile()
    return nc


def _prep_shared(W_kv, W_gate, ape, norm_w, H):
    W = np.concatenate([W_kv, W_gate], axis=0).astype(np.float32)  # [2048, 4096]
    Wb = W.astype(BF16)
    wp = np.ascontiguousarray(
        Wb.T.reshape(DCH, 128, OCHK, 128).transpose(2, 1, 0, 3))  # [16,128,32,128]
    ape_t = np.ascontiguousarray(
        ape.astype(np.float32).T.reshape(8, 128, 4).transpose(1, 0, 2)
    ).reshape(128, 32)
    psw = np.zeros((128, 128), np.float32)
    idx = np.arange(64)
    psw[idx, idx] = 1.0
    k2 = np.arange(0, 64, 2)
    psw[64 + k2 + 1, 64 + k2] = 1.0
    psw[64 + k2, 64 + k2 + 1] = 1.0
    hm = np.ascontiguousarray(
        (norm_w.astype(np.float32)[:, None] * H.astype(np.float32))
        .reshape(4, 128, 512).transpose(1, 0, 2)).astype(BF16)
    return wp, ape_t, psw.astype(BF16), hm


def _hadamard(n):
    h = np.array([[1.0]], dtype=np.float32)
    while h.shape[0] < n:
        h = np.block([[h, h], [h, -h]])
    return (h / np.sqrt(n)).astype(np.float32)


def _make_in_maps(x, W_kv, W_gate, ape, norm_w, freqs_cis):
    b, s, _ = x.shape
    H = _hadamard(512)
    wp, ape_t, psw, hm = _prep_shared(W_kv, W_gate, ape, norm_w, H)

    # truncate-to-bf16 (hi-16 planes of the f32 words) and transpose once
    xh = x.reshape(b * s, DIM).view(BF16)[:, 1::2]
    xT = np.ascontiguousarray(xh.T)  # [4096, 16384]
    fr = freqs_cis[:, :, 0]  # [nb, 32]
    fi = freqs_cis[:, :, 1]

    in_maps = []
    for c in range(N_CORES):
        batch, half = c // 2, c % 2
        R0 = batch * s + half * ROWS
        xs = np.zeros((DIM, XS_ROWS), BF16)
        xs[:, 16:] = xT[:, R0:R0 + ROWS]
        if half == 1:
            xs[:, :16] = xT[:, R0 - 16:R0]

        g0 = half * 512
        bi = np.arange(g0, g0 + 512)
        cos_t = np.zeros((128, 512), np.float32)
        cos_t[:64] = 1.0
        cos_t[64:] = np.repeat(fr[bi].T, 2, axis=0)
        sin_t = np.zeros((128, 512), np.float32)
        st = np.repeat(fi[bi].T, 2, axis=0)
        st[0::2] *= -1.0
        sin_t[64:] = st

        zmask = np.full((128, 1), 0.0 if half == 0 else 1.0, np.float32)
        in_maps.append({
            "xs": xs, "wp": wp, "ape_t": ape_t,
            "cos_t": cos_t, "sin_t": sin_t, "psw": psw,
            "hmat": hm, "zmask": zmask,
        })
    return in_maps


def kernel(x, W_kv, W_gate, ape, norm_w, freqs_cis, start_pos=0):
    x = np.asarray(x, dtype=np.float32)
    W_kv = np.asarray(W_kv, dtype=np.float32)
    W_gate = np.asarray(W_gate, dtype=np.float32)
    ape = np.asarray(ape, dtype=np.float32)
    norm_w = np.asarray(norm_w, dtype=np.float32)
    freqs_cis = np.asarray(freqs_cis, dtype=np.float32)

    b, s, _ = x.shape
    nb = s // 4
    assert (b, s) == (4, 4096), (b, s)

    if "nc" not in _CACHE:
        _CACHE["nc"] = _build()
    nc = _CACHE["nc"]

    in_maps = _make_in_maps(x, W_kv, W_gate, ape, norm_w, freqs_cis)

    trace = os.environ.get("KERNEL_TRACE", "") not in ("", "0")
    res = run_bass_kernel_spmd(nc, in_maps, core_ids=list(range(N_CORES)),
                               trace=trace)
    kernel.last_results = res
    out = np.concatenate([res.results[c]["out"] for c in range(N_CORES)], axis=0)
    return np.ascontiguousarray(out.reshape(b, nb, 512))
